# revision 3
# baseline (speedup 1.0000x reference)
"""Adaptive-softmax logits kernel for one TRN2 chip (8 NeuronCores).

Strategy
--------
reference computes three outputs from hidden (4096x1024):
  head    = hidden @ concat(candidates[:2000], tail_vectors).T   (4096, 2002)
  logits0 = ((hidden @ down0_w.T) @ candidates[2000:20000].T) * mask0
  logits1 = ((hidden @ down1_w.T) @ decode1_w.T) * mask1
mask0/mask1 zero out rows whose target is outside the bucket, so only the
~18% / ~80% of rows in each bucket are ever nonzero.  The host compacts the
bucket rows (gather by target), the device computes dense compact GEMMs, and
the host scatters rows back into zero-filled full outputs.

Sharding: head is batch-sharded (512 rows/core); the two tail GEMMs are
vocab-sharded (2250 / ~10001 columns per core).  The small down-projections
are replicated.  No collectives; the 8 cores run one SPMD NEFF.

Compute dtype bf16 (fp32 PSUM accumulate), outputs stored bf16 and upcast on
host.  All operands are pre-transposed on the host so both matmul operands
have the contraction dim on partitions.
"""

import json
import math

import ml_dtypes
import numpy as np

N_CORES = 8
C0, C1V, C2V = 2000, 20000, 100001
P = 128  # partitions / contraction tile
NT = 512  # moving-operand / psum free-dim tile

_BF16 = ml_dtypes.bfloat16


def _fix_json_bytes(js: bytes, maxw: int = 1) -> bytes:
    """Split >maxw per-instruction sem waits onto preceding same-engine NoOps.

    The walrus build in this image rejects instructions with more than one
    sync-wait command; Tile freely attaches several.  Program order within an
    engine makes hoisting the extra waits onto NoOps semantically identical.
    """
    d = json.loads(js)
    n_split = 0
    for fn in d["functions"]:
        for bb in fn["blocks"]:
            out = []
            for inst in bb["instructions"]:
                si = inst.get("sync_info")
                waits = (si or {}).get("on_wait") or []
                if len(waits) > maxw:
                    excess = waits[: len(waits) - maxw]
                    si["on_wait"] = waits[len(waits) - maxw :]
                    while excess:
                        chunk, excess = excess[:maxw], excess[maxw:]
                        n_split += 1
                        out.append(
                            {
                                "debug": inst.get("debug", 0),
                                "engine": inst["engine"],
                                "ins": [],
                                "outs": [],
                                "name": f"I-wsplit-{n_split}",
                                "opcode": "NoOp",
                                "text_hint": "waitsplit",
                                "sync_info": {"on_update": [], "on_wait": chunk},
                            }
                        )
                out.append(inst)
            bb["instructions"] = out
    return json.dumps(d).encode() if n_split else js


def _install_waitfix(nc):
    orig = nc.to_json_bytes
    nc.to_json_bytes = lambda: _fix_json_bytes(orig())


def _ntiles(total, tile):
    return [
        (i * tile, min(tile, total - i * tile)) for i in range(math.ceil(total / tile))
    ]


def build_graph(B, H, HR, BS, NH, W0, W1, N0P, N1P):
    """Build the single-core Bass graph (shared SPMD across the 8 cores)."""
    import concourse.bass as bass
    import concourse.mybir as mybir
    from concourse import tile

    bf = mybir.dt.bfloat16
    f32 = mybir.dt.float32
    KH = H // P  # 8 k-tiles over hidden dim
    KR = HR // P  # 2 k-tiles over reduced dim

    nc = bass.Bass("TRN2", target_bir_lowering=False, debug=False, num_devices=N_CORES)

    hth = nc.declare_dram_parameter("hth", [H, BS], bf, isOutput=False)
    h0t = nc.declare_dram_parameter("h0t", [H, N0P], bf, isOutput=False)
    h1t = nc.declare_dram_parameter("h1t", [H, N1P], bf, isOutput=False)
    hwt = nc.declare_dram_parameter("hwt", [H, NH], bf, isOutput=False)
    c0t = nc.declare_dram_parameter("c0t", [H, W0], bf, isOutput=False)
    d0t = nc.declare_dram_parameter("d0t", [H, H], bf, isOutput=False)
    d1t = nc.declare_dram_parameter("d1t", [H, HR], bf, isOutput=False)
    e1t = nc.declare_dram_parameter("e1t", [HR, W1], bf, isOutput=False)
    head_o = nc.declare_dram_parameter("head", [BS, NH], bf, isOutput=True)
    o0 = nc.declare_dram_parameter("o0", [N0P, W0], bf, isOutput=True)
    o1 = nc.declare_dram_parameter("o1", [N1P, W1], bf, isOutput=True)

    def kview(ap, k):  # (K*P, N) dram view -> [P, k, N]
        return ap[:].rearrange("(k p) n -> p k n", p=P)

    cp_cnt = [0]

    def cast_copy(out_ap, in_ap):
        cp_cnt[0] += 1
        if cp_cnt[0] % 2:
            nc.vector.tensor_copy(out_ap, in_ap)
        else:
            nc.scalar.copy(out_ap, in_ap)

    with tile.TileContext(nc) as tc:
        with (
            tc.tile_pool(name="const", bufs=1) as const,
            tc.tile_pool(name="wstream", bufs=4) as wstream,
            tc.tile_pool(name="d0stream", bufs=2) as d0stream,
            tc.tile_pool(name="obuf", bufs=2) as obuf,
            tc.tile_pool(name="psum", bufs=8, space="PSUM") as psum,
        ):
            # resident loads
            e1t_sb = const.tile([P, KR, W1], bf, tag="e1t")
            nc.sync.dma_start(e1t_sb[:], kview(e1t, KR))
            d1t_sb = const.tile([P, KH, HR], bf, tag="d1t")
            nc.sync.dma_start(d1t_sb[:], kview(d1t, KH))
            h0t_sb = const.tile([P, KH, N0P], bf, tag="h0t")
            nc.sync.dma_start(h0t_sb[:], kview(h0t, KH))
            hth_sb = const.tile([P, KH, BS], bf, tag="hth")
            nc.sync.dma_start(hth_sb[:], kview(hth, KH))
            h1a = const.tile([P, KR, N1P], bf, tag="h1a")
            h0a = const.tile([P, KH, N0P], bf, tag="h0a")

            # ---- phase A1: h1a[d, tok] = down1_w @ hidden1T (M=HR, N=N1P, K=H)
            for n0_, nsz in _ntiles(N1P, NT):
                ht = wstream.tile([P, KH, NT], bf, tag="wst")
                nc.sync.dma_start(
                    ht[:, :, :nsz], kview(h1t, KH)[:, :, n0_ : n0_ + nsz]
                )
                for mi in range(KR):
                    ps = psum.tile([P, NT], f32, tag="ps")
                    for k in range(KH):
                        nc.tensor.matmul(
                            ps[:, :nsz],
                            d1t_sb[:, k, mi * P : (mi + 1) * P],
                            ht[:, k, :nsz],
                            start=(k == 0),
                            stop=(k == KH - 1),
                        )
                    cast_copy(h1a[:, mi, n0_ : n0_ + nsz], ps[:, :nsz])

            # ---- phase C: o1[tok, v] = h1a.T @ e1t   (M=N1P, N=W1, K=HR)
            n1_tiles = _ntiles(W1, NT)
            half = (len(n1_tiles) + 3) // 4  # 4 output-buffer chunks per row
            chunks = [n1_tiles[i : i + half] for i in range(0, len(n1_tiles), half)]
            for m in range(N1P // P):
                for chunk in chunks:
                    c_lo = chunk[0][0]
                    c_hi = chunk[-1][0] + chunk[-1][1]
                    ob = obuf.tile([P, chunks[0][-1][0] + chunks[0][-1][1]], bf, tag="o1b")
                    for n0_, nsz in chunk:
                        ps = psum.tile([P, NT], f32, tag="ps")
                        for k in range(KR):
                            nc.tensor.matmul(
                                ps[:, :nsz],
                                h1a[:, k, m * P : (m + 1) * P],
                                e1t_sb[:, k, n0_ : n0_ + nsz],
                                start=(k == 0),
                                stop=(k == KR - 1),
                            )
                        cast_copy(ob[:, n0_ - c_lo : n0_ - c_lo + nsz], ps[:, :nsz])
                    nc.sync.dma_start(
                        o1[m * P : (m + 1) * P, c_lo:c_hi], ob[:, : c_hi - c_lo]
                    )

            # ---- phase A0: h0a[d, tok] = down0_w @ hidden0T (M=H, N=N0P, K=H)
            for mi in range(KH):
                dt_ = d0stream.tile([P, KH, P], bf, tag="d0t")
                nc.sync.dma_start(dt_[:], kview(d0t, KH)[:, :, mi * P : (mi + 1) * P])
                for n0_, nsz in _ntiles(N0P, NT):
                    ps = psum.tile([P, NT], f32, tag="ps")
                    for k in range(KH):
                        nc.tensor.matmul(
                            ps[:, :nsz],
                            dt_[:, k, :],
                            h0t_sb[:, k, n0_ : n0_ + nsz],
                            start=(k == 0),
                            stop=(k == KH - 1),
                        )
                    cast_copy(h0a[:, mi, n0_ : n0_ + nsz], ps[:, :nsz])

            # ---- phase B: o0[tok, v] = h0a.T @ c0t  (M=N0P, N=W0, K=H)
            for m in range(N0P // P):
                ob = obuf.tile([P, W0], bf, tag="o0b")
                for n0_, nsz in _ntiles(W0, NT):
                    ct = wstream.tile([P, KH, NT], bf, tag="wst")
                    nc.sync.dma_start(
                        ct[:, :, :nsz], kview(c0t, KH)[:, :, n0_ : n0_ + nsz]
                    )
                    ps = psum.tile([P, NT], f32, tag="ps")
                    for k in range(KH):
                        nc.tensor.matmul(
                            ps[:, :nsz],
                            h0a[:, k, m * P : (m + 1) * P],
                            ct[:, k, :nsz],
                            start=(k == 0),
                            stop=(k == KH - 1),
                        )
                    cast_copy(ob[:, n0_ : n0_ + nsz], ps[:, :nsz])
                nc.sync.dma_start(o0[m * P : (m + 1) * P, :], ob[:])

            # ---- head: head[tok, v] = hth.T @ hwt  (M=BS, N=NH, K=H)
            for m in range(BS // P):
                ob = obuf.tile([P, NH], bf, tag="hdb")
                for n0_, nsz in _ntiles(NH, NT):
                    wt = wstream.tile([P, KH, NT], bf, tag="wst")
                    nc.sync.dma_start(
                        wt[:, :, :nsz], kview(hwt, KH)[:, :, n0_ : n0_ + nsz]
                    )
                    ps = psum.tile([P, NT], f32, tag="ps")
                    for k in range(KH):
                        nc.tensor.matmul(
                            ps[:, :nsz],
                            hth_sb[:, k, m * P : (m + 1) * P],
                            wt[:, k, :nsz],
                            start=(k == 0),
                            stop=(k == KH - 1),
                        )
                    cast_copy(ob[:, n0_ : n0_ + nsz], ps[:, :nsz])
                nc.sync.dma_start(head_o[m * P : (m + 1) * P, :], ob[:])

    _install_waitfix(nc)
    return nc


def prepare(hidden, targets, candidates, tail_vectors, down0_w, down1_w, decode1_w):
    """Host-side shard/compact/transpose; returns (dims, in_maps, idx0, idx1)."""
    hidden = np.asarray(hidden, dtype=np.float32)
    targets = np.asarray(targets)
    candidates = np.asarray(candidates, dtype=np.float32)
    tail_vectors = np.asarray(tail_vectors, dtype=np.float32)
    down0_w = np.asarray(down0_w, dtype=np.float32)
    down1_w = np.asarray(down1_w, dtype=np.float32)
    decode1_w = np.asarray(decode1_w, dtype=np.float32)

    B, H = hidden.shape
    HR = down1_w.shape[0]
    BS = B // N_CORES
    NH = C0 + tail_vectors.shape[0]
    V0 = C1V - C0
    V1 = C2V - C1V
    W0 = V0 // N_CORES
    W1 = math.ceil(V1 / N_CORES)

    idx0 = np.nonzero((targets >= C0) & (targets < C1V))[0]
    idx1 = np.nonzero((targets >= C1V) & (targets < C2V))[0]
    n0, n1 = len(idx0), len(idx1)
    N0P = max(P, math.ceil(n0 / P) * P)
    N1P = max(P, math.ceil(n1 / P) * P)

    hT = np.ascontiguousarray(hidden.T).astype(_BF16)  # (H, B)
    h0t = np.zeros((H, N0P), _BF16)
    h0t[:, :n0] = hT[:, idx0]
    h1t = np.zeros((H, N1P), _BF16)
    h1t[:, :n1] = hT[:, idx1]
    hwt = np.concatenate([candidates[:C0], tail_vectors], 0).T.astype(_BF16)
    d0t = down0_w.T.astype(_BF16)
    d1t = down1_w.T.astype(_BF16)
    e1pad = np.zeros((N_CORES * W1, HR), np.float32)
    e1pad[:V1] = decode1_w

    shared = {"h0t": h0t, "h1t": h1t, "hwt": hwt, "d0t": d0t, "d1t": d1t}
    in_maps = []
    for c in range(N_CORES):
        m = dict(shared)
        m["hth"] = np.ascontiguousarray(hT[:, c * BS : (c + 1) * BS])
        m["c0t"] = np.ascontiguousarray(
            candidates[C0 + c * W0 : C0 + (c + 1) * W0].T
        ).astype(_BF16)
        m["e1t"] = np.ascontiguousarray(e1pad[c * W1 : (c + 1) * W1].T).astype(_BF16)
        in_maps.append(m)

    dims = dict(B=B, H=H, HR=HR, BS=BS, NH=NH, W0=W0, W1=W1, N0P=N0P, N1P=N1P)
    return dims, in_maps, idx0, idx1, n0, n1


def assemble(results, dims, idx0, idx1, n0, n1):
    B = dims["B"]
    V0 = C1V - C0
    V1 = C2V - C1V
    BS, NH, W0, W1 = dims["BS"], dims["NH"], dims["W0"], dims["W1"]

    head = np.empty((B, NH), np.float32)
    logits0 = np.zeros((B, V0), np.float32)
    logits1 = np.zeros((B, V1), np.float32)
    for c in range(N_CORES):
        r = results[c]
        head[c * BS : (c + 1) * BS] = r["head"].astype(np.float32)
        if n0:
            logits0[idx0, c * W0 : (c + 1) * W0] = r["o0"][:n0].astype(np.float32)
        if n1:
            w = min(W1, V1 - c * W1)
            logits1[idx1, c * W1 : c * W1 + w] = r["o1"][:n1, :w].astype(np.float32)
    return head, logits0, logits1


def run(inputs, trace=False, trace_kwargs=None):
    from concourse.bass_utils import run_bass_kernel_spmd

    dims, in_maps, idx0, idx1, n0, n1 = prepare(**inputs)
    nc = build_graph(
        dims["B"], dims["H"], dims["HR"], dims["BS"], dims["NH"],
        dims["W0"], dims["W1"], dims["N0P"], dims["N1P"],
    )
    res = run_bass_kernel_spmd(
        nc,
        in_maps,
        list(range(N_CORES)),
        trace=trace,
        **(trace_kwargs or {}),
    )
    outs = assemble(res.results, dims, idx0, idx1, n0, n1)
    return outs, res


def kernel(**inputs):
    outs, _ = run(inputs, trace=False)
    return outs


# revision 4
# speedup vs baseline: 1.0571x; 1.0571x over previous
"""Adaptive-softmax logits kernel for one TRN2 chip (8 NeuronCores).

Strategy
--------
reference computes three outputs from hidden (4096x1024):
  head    = hidden @ concat(candidates[:2000], tail_vectors).T   (4096, 2002)
  logits0 = ((hidden @ down0_w.T) @ candidates[2000:20000].T) * mask0
  logits1 = ((hidden @ down1_w.T) @ decode1_w.T) * mask1
mask0/mask1 zero out rows whose target is outside the bucket, so only the
~18% / ~80% of rows in each bucket are ever nonzero.  The host compacts the
bucket rows (gather by target), the device computes dense compact GEMMs, and
the host scatters rows back into zero-filled full outputs.

Sharding: head is batch-sharded (512 rows/core); the two tail GEMMs are
vocab-sharded (2250 / ~10001 columns per core).  The small down-projections
are replicated.  No collectives; the 8 cores run one SPMD NEFF.

Compute dtype bf16 (fp32 PSUM accumulate), outputs stored bf16 and upcast on
host.  All operands are pre-transposed on the host so both matmul operands
have the contraction dim on partitions.
"""

import json
import math

import ml_dtypes
import numpy as np

N_CORES = 8
C0, C1V, C2V = 2000, 20000, 100001
P = 128  # partitions / contraction tile
NT = 512  # moving-operand / psum free-dim tile

_BF16 = ml_dtypes.bfloat16


def _fix_json_bytes(js: bytes, maxw: int = 1) -> bytes:
    """Split >maxw per-instruction sem waits onto preceding same-engine NoOps.

    The walrus build in this image rejects instructions with more than one
    sync-wait command; Tile freely attaches several.  Program order within an
    engine makes hoisting the extra waits onto NoOps semantically identical.
    """
    d = json.loads(js)
    n_split = 0
    for fn in d["functions"]:
        for bb in fn["blocks"]:
            out = []
            for inst in bb["instructions"]:
                si = inst.get("sync_info")
                waits = (si or {}).get("on_wait") or []
                if len(waits) > maxw:
                    excess = waits[: len(waits) - maxw]
                    si["on_wait"] = waits[len(waits) - maxw :]
                    while excess:
                        chunk, excess = excess[:maxw], excess[maxw:]
                        n_split += 1
                        out.append(
                            {
                                "debug": inst.get("debug", 0),
                                "engine": inst["engine"],
                                "ins": [],
                                "outs": [],
                                "name": f"I-wsplit-{n_split}",
                                "opcode": "NoOp",
                                "text_hint": "waitsplit",
                                "sync_info": {"on_update": [], "on_wait": chunk},
                            }
                        )
                out.append(inst)
            bb["instructions"] = out
    return json.dumps(d).encode() if n_split else js


def _install_waitfix(nc):
    orig = nc.to_json_bytes
    nc.to_json_bytes = lambda: _fix_json_bytes(orig())


def _ntiles(total, tile):
    return [
        (i * tile, min(tile, total - i * tile)) for i in range(math.ceil(total / tile))
    ]


def build_graph(B, H, HR, BS, NH, W0, W1, N0P, N1P):
    """Build the single-core Bass graph (shared SPMD across the 8 cores)."""
    import concourse.bass as bass
    import concourse.mybir as mybir
    from concourse import tile

    bf = mybir.dt.bfloat16
    f32 = mybir.dt.float32
    KH = H // P  # 8 k-tiles over hidden dim
    KR = HR // P  # 2 k-tiles over reduced dim

    nc = bass.Bass("TRN2", target_bir_lowering=False, debug=False, num_devices=N_CORES)

    hth = nc.declare_dram_parameter("hth", [H, BS], bf, isOutput=False)
    h0t = nc.declare_dram_parameter("h0t", [H, N0P], bf, isOutput=False)
    h1t = nc.declare_dram_parameter("h1t", [H, N1P], bf, isOutput=False)
    hwt = nc.declare_dram_parameter("hwt", [H, NH], bf, isOutput=False)
    c0t = nc.declare_dram_parameter("c0t", [H, W0], bf, isOutput=False)
    d0t = nc.declare_dram_parameter("d0t", [H, H], bf, isOutput=False)
    d1t = nc.declare_dram_parameter("d1t", [H, HR], bf, isOutput=False)
    e1t = nc.declare_dram_parameter("e1t", [HR, W1], bf, isOutput=False)
    head_o = nc.declare_dram_parameter("head", [BS, NH], bf, isOutput=True)
    o0 = nc.declare_dram_parameter("o0", [N0P, W0], bf, isOutput=True)
    o1 = nc.declare_dram_parameter("o1", [N1P, W1], bf, isOutput=True)

    def kview(ap, k):  # (K*P, N) dram view -> [P, k, N]
        return ap[:].rearrange("(k p) n -> p k n", p=P)

    cp_cnt = [0]

    def cast_copy(out_ap, in_ap):
        cp_cnt[0] += 1
        if cp_cnt[0] % 2:
            nc.vector.tensor_copy(out_ap, in_ap)
        else:
            nc.scalar.copy(out_ap, in_ap)

    with tile.TileContext(nc) as tc:
        with (
            tc.tile_pool(name="const", bufs=1) as const,
            tc.tile_pool(name="wstream", bufs=3) as wstream,
            tc.tile_pool(name="d0stream", bufs=2) as d0stream,
            tc.tile_pool(name="obuf", bufs=2) as obuf,
            tc.tile_pool(name="psum", bufs=8, space="PSUM") as psum,
        ):
            # resident loads
            e1t_sb = const.tile([P, KR, W1], bf, tag="e1t")
            nc.sync.dma_start(e1t_sb[:], kview(e1t, KR))
            d1t_sb = const.tile([P, KH, HR], bf, tag="d1t")
            nc.sync.dma_start(d1t_sb[:], kview(d1t, KH))
            h0t_sb = const.tile([P, KH, N0P], bf, tag="h0t")
            nc.sync.dma_start(h0t_sb[:], kview(h0t, KH))
            hth_sb = const.tile([P, KH, BS], bf, tag="hth")
            nc.sync.dma_start(hth_sb[:], kview(hth, KH))
            h1a = const.tile([P, KR, N1P], bf, tag="h1a")
            h0a = const.tile([P, KH, N0P], bf, tag="h0a")

            # ---- phase A1: h1a[d, tok] = down1_w @ hidden1T (M=HR, N=N1P, K=H)
            for n0_, nsz in _ntiles(N1P, NT):
                ht = wstream.tile([P, KH, NT], bf, tag="wst")
                nc.sync.dma_start(
                    ht[:, :, :nsz], kview(h1t, KH)[:, :, n0_ : n0_ + nsz]
                )
                for mi in range(KR):
                    ps = psum.tile([P, NT], f32, tag="ps")
                    for k in range(KH):
                        nc.tensor.matmul(
                            ps[:, :nsz],
                            d1t_sb[:, k, mi * P : (mi + 1) * P],
                            ht[:, k, :nsz],
                            start=(k == 0),
                            stop=(k == KH - 1),
                        )
                    cast_copy(h1a[:, mi, n0_ : n0_ + nsz], ps[:, :nsz])

            # ---- phase A0: h0a[d, tok] = down0_w @ hidden0T (M=H, N=N0P, K=H)
            for mi in range(KH):
                dt_ = d0stream.tile([P, KH, P], bf, tag="d0t")
                nc.sync.dma_start(dt_[:], kview(d0t, KH)[:, :, mi * P : (mi + 1) * P])
                for n0_, nsz in _ntiles(N0P, NT):
                    ps = psum.tile([P, NT], f32, tag="ps")
                    for k in range(KH):
                        nc.tensor.matmul(
                            ps[:, :nsz],
                            dt_[:, k, :],
                            h0t_sb[:, k, n0_ : n0_ + nsz],
                            start=(k == 0),
                            stop=(k == KH - 1),
                        )
                    cast_copy(h0a[:, mi, n0_ : n0_ + nsz], ps[:, :nsz])

            # ---- head: head[tok, v] = hth.T @ hwt  (M=BS, N=NH, K=H)
            for m in range(BS // P):
                ob = obuf.tile([P, NH], bf, tag="hdb")
                for n0_, nsz in _ntiles(NH, NT):
                    wt = wstream.tile([P, KH, NT], bf, tag="hwst")
                    nc.sync.dma_start(
                        wt[:, :, :nsz], kview(hwt, KH)[:, :, n0_ : n0_ + nsz]
                    )
                    ps = psum.tile([P, NT], f32, tag="ps")
                    for k in range(KH):
                        nc.tensor.matmul(
                            ps[:, :nsz],
                            hth_sb[:, k, m * P : (m + 1) * P],
                            wt[:, k, :nsz],
                            start=(k == 0),
                            stop=(k == KH - 1),
                        )
                    cast_copy(ob[:, n0_ : n0_ + nsz], ps[:, :nsz])
                nc.sync.dma_start(head_o[m * P : (m + 1) * P, :], ob[:])

            # ---- phase B: o0[tok, v] = h0a.T @ c0t  (M=N0P, N=W0, K=H)
            for m in range(N0P // P):
                ob = obuf.tile([P, W0], bf, tag="o0b")
                for n0_, nsz in _ntiles(W0, NT):
                    ct = wstream.tile([P, KH, NT], bf, tag="c0st")
                    nc.sync.dma_start(
                        ct[:, :, :nsz], kview(c0t, KH)[:, :, n0_ : n0_ + nsz]
                    )
                    ps = psum.tile([P, NT], f32, tag="ps")
                    for k in range(KH):
                        nc.tensor.matmul(
                            ps[:, :nsz],
                            h0a[:, k, m * P : (m + 1) * P],
                            ct[:, k, :nsz],
                            start=(k == 0),
                            stop=(k == KH - 1),
                        )
                    cast_copy(ob[:, n0_ : n0_ + nsz], ps[:, :nsz])
                nc.sync.dma_start(o0[m * P : (m + 1) * P, :], ob[:])

            # ---- phase C: o1[tok, v] = h1a.T @ e1t   (M=N1P, N=W1, K=HR)
            n1_tiles = _ntiles(W1, NT)
            half = (len(n1_tiles) + 3) // 4  # 4 output-buffer chunks per row
            chunks = [n1_tiles[i : i + half] for i in range(0, len(n1_tiles), half)]
            for m in range(N1P // P):
                for chunk in chunks:
                    c_lo = chunk[0][0]
                    c_hi = chunk[-1][0] + chunk[-1][1]
                    ob = obuf.tile([P, chunks[0][-1][0] + chunks[0][-1][1]], bf, tag="o1b")
                    for n0_, nsz in chunk:
                        ps = psum.tile([P, NT], f32, tag="ps")
                        for k in range(KR):
                            nc.tensor.matmul(
                                ps[:, :nsz],
                                h1a[:, k, m * P : (m + 1) * P],
                                e1t_sb[:, k, n0_ : n0_ + nsz],
                                start=(k == 0),
                                stop=(k == KR - 1),
                            )
                        cast_copy(ob[:, n0_ - c_lo : n0_ - c_lo + nsz], ps[:, :nsz])
                    nc.sync.dma_start(
                        o1[m * P : (m + 1) * P, c_lo:c_hi], ob[:, : c_hi - c_lo]
                    )

    _install_waitfix(nc)
    return nc


def prepare(hidden, targets, candidates, tail_vectors, down0_w, down1_w, decode1_w):
    """Host-side shard/compact/transpose; returns (dims, in_maps, idx0, idx1)."""
    hidden = np.asarray(hidden, dtype=np.float32)
    targets = np.asarray(targets)
    candidates = np.asarray(candidates, dtype=np.float32)
    tail_vectors = np.asarray(tail_vectors, dtype=np.float32)
    down0_w = np.asarray(down0_w, dtype=np.float32)
    down1_w = np.asarray(down1_w, dtype=np.float32)
    decode1_w = np.asarray(decode1_w, dtype=np.float32)

    B, H = hidden.shape
    HR = down1_w.shape[0]
    BS = B // N_CORES
    NH = C0 + tail_vectors.shape[0]
    V0 = C1V - C0
    V1 = C2V - C1V
    W0 = V0 // N_CORES
    W1 = math.ceil(V1 / N_CORES)

    idx0 = np.nonzero((targets >= C0) & (targets < C1V))[0]
    idx1 = np.nonzero((targets >= C1V) & (targets < C2V))[0]
    n0, n1 = len(idx0), len(idx1)
    N0P = max(P, math.ceil(n0 / P) * P)
    N1P = max(P, math.ceil(n1 / P) * P)

    hT = np.ascontiguousarray(hidden.T).astype(_BF16)  # (H, B)
    h0t = np.zeros((H, N0P), _BF16)
    h0t[:, :n0] = hT[:, idx0]
    h1t = np.zeros((H, N1P), _BF16)
    h1t[:, :n1] = hT[:, idx1]
    hwt = np.concatenate([candidates[:C0], tail_vectors], 0).T.astype(_BF16)
    d0t = down0_w.T.astype(_BF16)
    d1t = down1_w.T.astype(_BF16)
    e1pad = np.zeros((N_CORES * W1, HR), np.float32)
    e1pad[:V1] = decode1_w

    shared = {"h0t": h0t, "h1t": h1t, "hwt": hwt, "d0t": d0t, "d1t": d1t}
    in_maps = []
    for c in range(N_CORES):
        m = dict(shared)
        m["hth"] = np.ascontiguousarray(hT[:, c * BS : (c + 1) * BS])
        m["c0t"] = np.ascontiguousarray(
            candidates[C0 + c * W0 : C0 + (c + 1) * W0].T
        ).astype(_BF16)
        m["e1t"] = np.ascontiguousarray(e1pad[c * W1 : (c + 1) * W1].T).astype(_BF16)
        in_maps.append(m)

    dims = dict(B=B, H=H, HR=HR, BS=BS, NH=NH, W0=W0, W1=W1, N0P=N0P, N1P=N1P)
    return dims, in_maps, idx0, idx1, n0, n1


def assemble(results, dims, idx0, idx1, n0, n1):
    B = dims["B"]
    V0 = C1V - C0
    V1 = C2V - C1V
    BS, NH, W0, W1 = dims["BS"], dims["NH"], dims["W0"], dims["W1"]

    head = np.empty((B, NH), np.float32)
    logits0 = np.zeros((B, V0), np.float32)
    logits1 = np.zeros((B, V1), np.float32)
    for c in range(N_CORES):
        r = results[c]
        head[c * BS : (c + 1) * BS] = r["head"].astype(np.float32)
        if n0:
            logits0[idx0, c * W0 : (c + 1) * W0] = r["o0"][:n0].astype(np.float32)
        if n1:
            w = min(W1, V1 - c * W1)
            logits1[idx1, c * W1 : c * W1 + w] = r["o1"][:n1, :w].astype(np.float32)
    return head, logits0, logits1


def run(inputs, trace=False, trace_kwargs=None):
    from concourse.bass_utils import run_bass_kernel_spmd

    dims, in_maps, idx0, idx1, n0, n1 = prepare(**inputs)
    nc = build_graph(
        dims["B"], dims["H"], dims["HR"], dims["BS"], dims["NH"],
        dims["W0"], dims["W1"], dims["N0P"], dims["N1P"],
    )
    res = run_bass_kernel_spmd(
        nc,
        in_maps,
        list(range(N_CORES)),
        trace=trace,
        **(trace_kwargs or {}),
    )
    outs = assemble(res.results, dims, idx0, idx1, n0, n1)
    return outs, res


def kernel(**inputs):
    outs, _ = run(inputs, trace=False)
    return outs


# revision 6
# speedup vs baseline: 1.4919x; 1.4113x over previous
"""Adaptive-softmax logits kernel for one TRN2 chip (8 NeuronCores).

Strategy
--------
reference computes three outputs from hidden (4096x1024):
  head    = hidden @ concat(candidates[:2000], tail_vectors).T   (4096, 2002)
  logits0 = ((hidden @ down0_w.T) @ candidates[2000:20000].T) * mask0
  logits1 = ((hidden @ down1_w.T) @ decode1_w.T) * mask1
mask0/mask1 zero out rows whose target is outside the bucket, so only the
~18% / ~80% of rows in each bucket are ever nonzero.  The host compacts the
bucket rows (gather by target), the device computes dense compact GEMMs, and
the host scatters rows back into zero-filled full outputs.

Sharding: head is batch-sharded (512 rows/core); the two tail GEMMs are
vocab-sharded (2250 / ~10001 columns per core).  The small down-projections
are replicated.  No collectives; the 8 cores run one SPMD NEFF.

Compute dtype bf16 (fp32 PSUM accumulate), outputs stored bf16 and upcast on
host.  All operands are pre-transposed on the host so both matmul operands
have the contraction dim on partitions.
"""

import json
import math

import ml_dtypes
import numpy as np

N_CORES = 8
C0, C1V, C2V = 2000, 20000, 100001
P = 128  # partitions / contraction tile
NT = 512  # moving-operand / psum free-dim tile

_BF16 = ml_dtypes.bfloat16


def _fix_json_bytes(js: bytes, maxw: int = 1) -> bytes:
    """Split >maxw per-instruction sem waits onto preceding same-engine NoOps.

    The walrus build in this image rejects instructions with more than one
    sync-wait command; Tile freely attaches several.  Program order within an
    engine makes hoisting the extra waits onto NoOps semantically identical.
    """
    d = json.loads(js)
    n_split = 0
    for fn in d["functions"]:
        for bb in fn["blocks"]:
            out = []
            for inst in bb["instructions"]:
                si = inst.get("sync_info")
                waits = (si or {}).get("on_wait") or []
                if len(waits) > maxw:
                    excess = waits[: len(waits) - maxw]
                    si["on_wait"] = waits[len(waits) - maxw :]
                    while excess:
                        chunk, excess = excess[:maxw], excess[maxw:]
                        n_split += 1
                        out.append(
                            {
                                "debug": inst.get("debug", 0),
                                "engine": inst["engine"],
                                "ins": [],
                                "outs": [],
                                "name": f"I-wsplit-{n_split}",
                                "opcode": "NoOp",
                                "text_hint": "waitsplit",
                                "sync_info": {"on_update": [], "on_wait": chunk},
                            }
                        )
                out.append(inst)
            bb["instructions"] = out
    return json.dumps(d).encode() if n_split else js


def _install_waitfix(nc):
    orig = nc.to_json_bytes
    nc.to_json_bytes = lambda: _fix_json_bytes(orig())


def _ntiles(total, tile):
    return [
        (i * tile, min(tile, total - i * tile)) for i in range(math.ceil(total / tile))
    ]


def build_graph(B, H, HR, BS, NH, W0, W1, N0P, N1P):
    """Build the single-core Bass graph (shared SPMD across the 8 cores)."""
    import concourse.bass as bass
    import concourse.mybir as mybir
    from concourse import tile

    bf = mybir.dt.bfloat16
    f32 = mybir.dt.float32
    KH = H // P  # 8 k-tiles over hidden dim
    KR = HR // P  # 2 k-tiles over reduced dim

    nc = bass.Bass("TRN2", target_bir_lowering=False, debug=False, num_devices=N_CORES)

    hth = nc.declare_dram_parameter("hth", [H, BS], bf, isOutput=False)
    h0t = nc.declare_dram_parameter("h0t", [H, N0P], bf, isOutput=False)
    h1t = nc.declare_dram_parameter("h1t", [H, N1P], bf, isOutput=False)
    hwt = nc.declare_dram_parameter("hwt", [H, NH], bf, isOutput=False)
    c0t = nc.declare_dram_parameter("c0t", [H, W0], bf, isOutput=False)
    d0t = nc.declare_dram_parameter("d0t", [H, H], bf, isOutput=False)
    d1t = nc.declare_dram_parameter("d1t", [H, HR], bf, isOutput=False)
    e1t = nc.declare_dram_parameter("e1t", [HR, W1], bf, isOutput=False)
    head_o = nc.declare_dram_parameter("head", [BS, NH], bf, isOutput=True)
    o0 = nc.declare_dram_parameter("o0", [N0P, W0], bf, isOutput=True)
    o1 = nc.declare_dram_parameter("o1", [N1P, W1], bf, isOutput=True)

    WH0 = (math.ceil(W1 / NT) // 2) * NT  # first-half vocab cols of the big decode
    WH1 = W1 - WH0

    def kview(ap, k):  # (K*P, N) dram view -> [P, k, N]
        return ap[:].rearrange("(k p) n -> p k n", p=P)

    cp_cnt = [0]

    def cast_copy(out_ap, in_ap):
        cp_cnt[0] += 1
        if cp_cnt[0] % 2:
            nc.vector.tensor_copy(out_ap, in_ap)
        else:
            nc.scalar.copy(out_ap, in_ap)

    with tile.TileContext(nc) as tc:
        with (
            tc.tile_pool(name="const", bufs=1) as const,
            tc.tile_pool(name="d0stream", bufs=2) as d0stream,
            tc.tile_pool(name="obuf", bufs=2) as obuf,
            tc.tile_pool(name="psum", bufs=4, space="PSUM") as psum,
        ):
            # resident loads (e1t first half prefetches early; used last)
            e1h0 = const.tile([P, KR, WH0], bf, tag="e1h0")
            nc.sync.dma_start(e1h0[:], kview(e1t, KR)[:, :, :WH0])
            d1t_sb = const.tile([P, KH, HR], bf, tag="d1t")
            nc.sync.dma_start(d1t_sb[:], kview(d1t, KH))
            h0t_sb = const.tile([P, KH, N0P], bf, tag="h0t")
            nc.sync.dma_start(h0t_sb[:], kview(h0t, KH))
            hth_sb = const.tile([P, KH, BS], bf, tag="hth")
            nc.sync.dma_start(hth_sb[:], kview(hth, KH))
            h1a = const.tile([P, KR, N1P], bf, tag="h1a")
            h0a = const.tile([P, KH, N0P], bf, tag="h0a")

            # ---- phase A1: h1a[d, tok] = down1_w @ hidden1T (M=HR, N=N1P, K=H)
            with tc.tile_pool(name="h1stream", bufs=3) as h1stream:
                for n0_, nsz in _ntiles(N1P, NT):
                    ht = h1stream.tile([P, KH, NT], bf, tag="wst")
                    nc.sync.dma_start(
                        ht[:, :, :nsz], kview(h1t, KH)[:, :, n0_ : n0_ + nsz]
                    )
                    for mi in range(KR):
                        ps = psum.tile([P, 2 * NT], f32, tag="ps")
                        for k in range(KH):
                            nc.tensor.matmul(
                                ps[:, :nsz],
                                d1t_sb[:, k, mi * P : (mi + 1) * P],
                                ht[:, k, :nsz],
                                start=(k == 0),
                                stop=(k == KH - 1),
                            )
                        cast_copy(h1a[:, mi, n0_ : n0_ + nsz], ps[:, :nsz])

                # ---- phase A0: h0a[d, tok] = down0_w @ hidden0T (M=H, N=N0P, K=H)
                for mi in range(KH):
                    dt_ = d0stream.tile([P, KH, P], bf, tag="d0t")
                    nc.sync.dma_start(
                        dt_[:], kview(d0t, KH)[:, :, mi * P : (mi + 1) * P]
                    )
                    for n0_, nsz in _ntiles(N0P, NT):
                        ps = psum.tile([P, 2 * NT], f32, tag="ps")
                        for k in range(KH):
                            nc.tensor.matmul(
                                ps[:, :nsz],
                                dt_[:, k, :],
                                h0t_sb[:, k, n0_ : n0_ + nsz],
                                start=(k == 0),
                                stop=(k == KH - 1),
                            )
                        cast_copy(h0a[:, mi, n0_ : n0_ + nsz], ps[:, :nsz])

            # ---- head + B with resident weights (prefetched as single DMAs)
            with tc.tile_pool(name="bh", bufs=1) as bh:
                hwt_sb = bh.tile([P, KH, NH], bf, tag="hwt")
                nc.sync.dma_start(hwt_sb[:], kview(hwt, KH))
                c0t_sb = bh.tile([P, KH, W0], bf, tag="c0t")
                nc.sync.dma_start(c0t_sb[:], kview(c0t, KH))

                # head: head[tok, v] = hth.T @ hwt  (M=BS, N=NH, K=H)
                for m in range(BS // P):
                    ob = obuf.tile([P, NH], bf, tag="hdb")
                    for n0_, nsz in _ntiles(NH, NT):
                        ps = psum.tile([P, 2 * NT], f32, tag="ps")
                        for k in range(KH):
                            nc.tensor.matmul(
                                ps[:, :nsz],
                                hth_sb[:, k, m * P : (m + 1) * P],
                                hwt_sb[:, k, n0_ : n0_ + nsz],
                                start=(k == 0),
                                stop=(k == KH - 1),
                            )
                        cast_copy(ob[:, n0_ : n0_ + nsz], ps[:, :nsz])
                    nc.sync.dma_start(head_o[m * P : (m + 1) * P, :], ob[:])

                # B: o0[tok, v] = h0a.T @ c0t  (M=N0P, N=W0, K=H)
                for m in range(N0P // P):
                    ob = obuf.tile([P, W0], bf, tag="o0b")
                    for n0_, nsz in _ntiles(W0, NT):
                        ps = psum.tile([P, 2 * NT], f32, tag="ps")
                        for k in range(KH):
                            nc.tensor.matmul(
                                ps[:, :nsz],
                                h0a[:, k, m * P : (m + 1) * P],
                                c0t_sb[:, k, n0_ : n0_ + nsz],
                                start=(k == 0),
                                stop=(k == KH - 1),
                            )
                        cast_copy(ob[:, n0_ : n0_ + nsz], ps[:, :nsz])
                    nc.sync.dma_start(o0[m * P : (m + 1) * P, :], ob[:])

            # ---- phase C: o1[tok, v] = h1a.T @ e1t  (M=N1P, N=W1, K=HR)
            with tc.tile_pool(name="e1late", bufs=1) as e1late:
                e1h1 = e1late.tile([P, KR, WH1], bf, tag="e1h1")
                nc.sync.dma_start(e1h1[:], kview(e1t, KR)[:, :, WH0:])

                for half, (esb, lo, wid) in enumerate(
                    [(e1h0, 0, WH0), (e1h1, WH0, WH1)]
                ):
                    tiles = _ntiles(wid, NT)
                    pairs = [tiles[i : i + 2] for i in range(0, len(tiles), 2)]
                    for m in range(N1P // P):
                        ob = obuf.tile([P, WH0], bf, tag="o1b")
                        for pair in pairs:
                            ps = psum.tile([P, 2 * NT], f32, tag="ps")
                            for j, (n0_, nsz) in enumerate(pair):
                                for k in range(KR):
                                    nc.tensor.matmul(
                                        ps[:, j * NT : j * NT + nsz],
                                        h1a[:, k, m * P : (m + 1) * P],
                                        esb[:, k, n0_ : n0_ + nsz],
                                        start=(k == 0),
                                        stop=(k == KR - 1),
                                    )
                            p_lo = pair[0][0]
                            p_w = pair[-1][0] + pair[-1][1] - p_lo
                            cast_copy(ob[:, p_lo : p_lo + p_w], ps[:, :p_w])
                        nc.sync.dma_start(
                            o1[m * P : (m + 1) * P, lo : lo + wid], ob[:, :wid]
                        )

    _install_waitfix(nc)
    return nc


def prepare(hidden, targets, candidates, tail_vectors, down0_w, down1_w, decode1_w):
    """Host-side shard/compact/transpose; returns (dims, in_maps, idx0, idx1)."""
    hidden = np.asarray(hidden, dtype=np.float32)
    targets = np.asarray(targets)
    candidates = np.asarray(candidates, dtype=np.float32)
    tail_vectors = np.asarray(tail_vectors, dtype=np.float32)
    down0_w = np.asarray(down0_w, dtype=np.float32)
    down1_w = np.asarray(down1_w, dtype=np.float32)
    decode1_w = np.asarray(decode1_w, dtype=np.float32)

    B, H = hidden.shape
    HR = down1_w.shape[0]
    BS = B // N_CORES
    NH = C0 + tail_vectors.shape[0]
    V0 = C1V - C0
    V1 = C2V - C1V
    W0 = V0 // N_CORES
    W1 = math.ceil(V1 / N_CORES)

    idx0 = np.nonzero((targets >= C0) & (targets < C1V))[0]
    idx1 = np.nonzero((targets >= C1V) & (targets < C2V))[0]
    n0, n1 = len(idx0), len(idx1)
    N0P = max(P, math.ceil(n0 / P) * P)
    N1P = max(P, math.ceil(n1 / P) * P)

    hT = np.ascontiguousarray(hidden.T).astype(_BF16)  # (H, B)
    h0t = np.zeros((H, N0P), _BF16)
    h0t[:, :n0] = hT[:, idx0]
    h1t = np.zeros((H, N1P), _BF16)
    h1t[:, :n1] = hT[:, idx1]
    hwt = np.concatenate([candidates[:C0], tail_vectors], 0).T.astype(_BF16)
    d0t = down0_w.T.astype(_BF16)
    d1t = down1_w.T.astype(_BF16)
    e1pad = np.zeros((N_CORES * W1, HR), np.float32)
    e1pad[:V1] = decode1_w

    shared = {"h0t": h0t, "h1t": h1t, "hwt": hwt, "d0t": d0t, "d1t": d1t}
    in_maps = []
    for c in range(N_CORES):
        m = dict(shared)
        m["hth"] = np.ascontiguousarray(hT[:, c * BS : (c + 1) * BS])
        m["c0t"] = np.ascontiguousarray(
            candidates[C0 + c * W0 : C0 + (c + 1) * W0].T
        ).astype(_BF16)
        m["e1t"] = np.ascontiguousarray(e1pad[c * W1 : (c + 1) * W1].T).astype(_BF16)
        in_maps.append(m)

    dims = dict(B=B, H=H, HR=HR, BS=BS, NH=NH, W0=W0, W1=W1, N0P=N0P, N1P=N1P)
    return dims, in_maps, idx0, idx1, n0, n1


def assemble(results, dims, idx0, idx1, n0, n1):
    B = dims["B"]
    V0 = C1V - C0
    V1 = C2V - C1V
    BS, NH, W0, W1 = dims["BS"], dims["NH"], dims["W0"], dims["W1"]

    head = np.empty((B, NH), np.float32)
    logits0 = np.zeros((B, V0), np.float32)
    logits1 = np.zeros((B, V1), np.float32)
    for c in range(N_CORES):
        r = results[c]
        head[c * BS : (c + 1) * BS] = r["head"].astype(np.float32)
        if n0:
            logits0[idx0, c * W0 : (c + 1) * W0] = r["o0"][:n0].astype(np.float32)
        if n1:
            w = min(W1, V1 - c * W1)
            logits1[idx1, c * W1 : c * W1 + w] = r["o1"][:n1, :w].astype(np.float32)
    return head, logits0, logits1


def run(inputs, trace=False, trace_kwargs=None):
    from concourse.bass_utils import run_bass_kernel_spmd

    dims, in_maps, idx0, idx1, n0, n1 = prepare(**inputs)
    nc = build_graph(
        dims["B"], dims["H"], dims["HR"], dims["BS"], dims["NH"],
        dims["W0"], dims["W1"], dims["N0P"], dims["N1P"],
    )
    res = run_bass_kernel_spmd(
        nc,
        in_maps,
        list(range(N_CORES)),
        trace=trace,
        **(trace_kwargs or {}),
    )
    outs = assemble(res.results, dims, idx0, idx1, n0, n1)
    return outs, res


def kernel(**inputs):
    outs, _ = run(inputs, trace=False)
    return outs
